# revision 60
# baseline (speedup 1.0000x reference)
"""Trainium2 Bass kernel for nn_BatchSplitFF (expert-choice MoE routing + FFN).

Strategy (data-parallel over batch, 1 batch per NeuronCore, 8 cores):
  - fp32 logits on PE in [es, tok] orientation, N=512 token chunks (routing
    must match the fp32 reference argmax; bf16 logits would flip decisions).
  - routing on DVE: group max -> one-hot iseq (== perm matrix in [es, tok]
    orientation) -> token ids exported for the host-side combine.
  - dispatch ON-CHIP via PE one-hot matmuls (no DMA gather: SWDGE descriptor
    overhead ~150us/queue dominated the old design): per 128-token tile,
    selT[dm, slot] = x_tile^T @ perm_tile. perm tiles are built on DVE from
    PE-transposed iseq slices. Two es-halves keep selT at 8MB in SBUF.
  - expert FFN in bf16 on PE with array tiling: up-proj packs es pairs into
    128x64 column tiles (2x), down-proj packs es pairs into 64x128 row tiles
    (2x). relu/bias on ScalarE.
  - un-permute: y rows are written DENSELY (one row per (es, g) contribution)
    with contiguous DMAs; the routing table (token id per contribution) is
    exported, and the host does the final scatter-add combine in fp32
    (8.4M adds = 0.04% of the FLOPs). dma_scatter_add loses duplicate-row
    updates (verified on HW) and on-chip combine requires a y transpose that
    costs as much as the dense write saves.
Host side only reshapes/casts/transposes inputs and combines the output.
"""

import numpy as np
import ml_dtypes

import concourse.bass as bass
import concourse.mybir as mybir
import concourse.tile as tile
from concourse import bacc
from concourse.bass_utils import run_bass_kernel_spmd

bf16 = ml_dtypes.bfloat16
f32 = mybir.dt.float32
f32r = mybir.dt.float32r
bfl = mybir.dt.bfloat16
i32 = mybir.dt.int32

DM, NE, ES, ESZ = 1024, 16, 4, 64
NES = NE * ES            # 64 (e,s) expert pairs
SEQ = 2048
G = SEQ // NE            # 128 groups per core
KT = DM // 128           # 8 contraction tiles
NCORES = 8
NTT = SEQ // 128         # 16 token tiles (8 groups each)
GPT = 8                  # groups per token tile
EHALF = NES // 2         # 32 es per dispatch half
NPAIR = NES // 2         # 32 es pairs
PPH = NPAIR // 2         # 16 pairs per half

_CACHE = {}


def _build_program():
    nc = bacc.Bacc("TRN2", target_bir_lowering=False, debug=False,
                   enable_asserts=False, num_devices=1)

    xTh = nc.dram_tensor("xTh", [DM, SEQ], bfl, kind="ExternalInput").ap()
    xTl = nc.dram_tensor("xTl", [DM, SEQ], bfl, kind="ExternalInput").ap()
    # natural-layout bf16 x wrapped [(r p) d -> p r d]: token t at
    # partition t%128, rank t//128 -> x_sb[:, tt, :] is a 128-token tile
    xbf = nc.dram_tensor("xbf", [128, NTT, DM], bfl, kind="ExternalInput").ap()
    c2h = nc.dram_tensor("c2h", [DM, NES], bfl, kind="ExternalInput").ap()
    c2l = nc.dram_tensor("c2l", [DM, NES], bfl, kind="ExternalInput").ap()
    f1w = nc.dram_tensor("f1w", [DM, NES * ESZ], bfl, kind="ExternalInput").ap()
    # f2 stacked in es pairs: [2*64 f, 32 pairs, DM]
    f2w = nc.dram_tensor("f2w", [128, NPAIR, DM], bfl, kind="ExternalInput").ap()
    # bias stacked in es pairs: [2*64 f, 32 pairs]
    bias2 = nc.dram_tensor("bias2", [128, NPAIR], f32, kind="ExternalInput").ap()
    tie512 = nc.dram_tensor("tie512", [NES, 512], f32, kind="ExternalInput").ap()
    tokid = nc.dram_tensor("tokid", [NES, SEQ], f32, kind="ExternalInput").ap()
    ident = nc.dram_tensor("ident", [NES, NES], f32, kind="ExternalInput").ap()
    # glmask[t%128, gl] = 1.0 iff (t%128)//16 == gl
    glmask = nc.dram_tensor("glmask", [128, GPT], bfl, kind="ExternalInput").ap()
    stage3 = nc.dram_tensor("stage3", [NES * G, DM], bfl,
                            kind="ExternalOutput").ap()
    tids = nc.dram_tensor("tids", [NES, G], i32, kind="ExternalOutput").ap()

    with tile.TileContext(nc) as tc:
        with (
            tc.tile_pool(name="consts", bufs=1) as consts,
            tc.tile_pool(name="route", bufs=1) as route,
        ):
            # ---- constants into SBUF ----
            ch_sb = consts.tile([128, KT, NES], bfl)
            nc.sync.dma_start(out=ch_sb[:], in_=c2h.rearrange("(k p) e -> p k e", p=128))
            cl_sb = consts.tile([128, KT, NES], bfl)
            nc.sync.dma_start(out=cl_sb[:], in_=c2l.rearrange("(k p) e -> p k e", p=128))
            tie_sb = consts.tile([NES, 512], f32)
            nc.sync.dma_start(out=tie_sb[:], in_=tie512)
            bias_sb = consts.tile([128, NPAIR], f32)
            tokid_sb = consts.tile([NES, SEQ], f32)
            ident_sb = consts.tile([NES, NES], f32)
            glmask_sb = consts.tile([128, GPT], bfl)
            x_sb = consts.tile([128, NTT, DM], bfl)

            # ---- phase B: fp32 logits, [es, tok], 512-token chunks ----
            # PE warmup: ~5us of dummy matmuls during the initial DMA wait
            # releases the HAM clock gate so real matmuls run at 2.4 GHz
            with tc.tile_pool(name="warm", bufs=1, space="PSUM") as warm:
                wps = warm.tile([NES, 512], f32, space="PSUM")
                wrhs = ch_sb.rearrange("p k e -> p (k e)")
                for _ in range(12):
                    nc.tensor.matmul(wps[:], ch_sb[:, 0, :], wrhs,
                                     start=True, stop=True)

            bc_cm = tc.tile_pool(name="bc", bufs=1)
            bc = bc_cm.__enter__()
            logits_sb = bc.tile([NES, SEQ], f32)
            iseq = bc.tile([NES, SEQ], f32)
            tsel = bc.tile([NES, SEQ], f32)
            perm_all = route.tile([128, NTT, NES, GPT], bfl)
            with (
                tc.tile_pool(name="xtp", bufs=3) as xtp,
                tc.tile_pool(name="rt", bufs=1) as rt,
                tc.tile_pool(name="psB", bufs=2, space="PSUM") as psB,
                tc.tile_pool(name="psC", bufs=4, space="PSUM") as psC,
            ):
                gmax = rt.tile([NES, G], f32)
                tid_f = rt.tile([NES, G], f32)
                xTh_r = xTh.rearrange("(k p) t -> p k t", p=128)
                xTl_r = xTl.rearrange("(k p) t -> p k t", p=128)
                xt_tiles = []
                for tc4 in range(SEQ // 512):
                    xt_t = xtp.tile([128, 2, KT, 512], bfl, tag="xt",
                                    name=f"xt_{tc4}")
                    nc.sync.dma_start(
                        out=xt_t[:, 0, :, :],
                        in_=xTh_r[:, :, tc4 * 512:(tc4 + 1) * 512])
                    nc.sync.dma_start(
                        out=xt_t[:, 1, :, :],
                        in_=xTl_r[:, :, tc4 * 512:(tc4 + 1) * 512])
                    xt_tiles.append(xt_t)
                    if tc4 == 0:
                        # deferred consts: queued after the first logits chunk
                        nc.sync.dma_start(out=tokid_sb[:], in_=tokid)
                        nc.sync.dma_start(out=ident_sb[:], in_=ident)
                        nc.sync.dma_start(out=glmask_sb[:], in_=glmask)
                        nc.sync.dma_start(out=bias_sb[:], in_=bias2)
                    if tc4 == 3:
                        # x_sb (dispatch source) queued after all logits
                        # chunks; needed only when dispatch starts
                        nc.sync.dma_start(out=x_sb[:], in_=xbf)
                # fused logits + routing, per 512-token chunk (32 groups)
                for tc4 in range(SEQ // 512):
                    xt_t = xt_tiles[tc4]
                    t0, t1 = tc4 * 512, (tc4 + 1) * 512
                    g0, g1 = tc4 * 32, (tc4 + 1) * 32
                    psum_l = psB.tile([NES, 512], f32, space="PSUM")
                    # fp32-split logits: xh*ch + xh*cl + xl*ch (bf16 full rate)
                    for k in range(KT):
                        nc.tensor.matmul(psum_l[:], ch_sb[:, k, :],
                                         xt_t[:, 0, k, :],
                                         start=(k == 0), stop=False)
                        nc.tensor.matmul(psum_l[:], cl_sb[:, k, :],
                                         xt_t[:, 0, k, :],
                                         start=False, stop=False)
                        nc.tensor.matmul(psum_l[:], ch_sb[:, k, :],
                                         xt_t[:, 1, k, :],
                                         start=False, stop=(k == KT - 1))
                    # add tiebreak while copying PSUM -> SBUF
                    nc.vector.tensor_tensor(
                        out=logits_sb[:, t0:t1],
                        in0=psum_l[:], in1=tie_sb[:],
                        op=mybir.AluOpType.add)
                    # routing for this chunk's 32 groups on DVE
                    lg = logits_sb[:, t0:t1].rearrange("e (g t) -> e g t", t=NE)
                    nc.vector.tensor_reduce(
                        out=gmax[:, g0:g1], in_=lg,
                        axis=mybir.AxisListType.X, op=mybir.AluOpType.max)
                    nc.vector.tensor_tensor(
                        out=iseq[:, t0:t1].rearrange("e (g t) -> e g t", t=NE),
                        in0=lg,
                        in1=gmax[:, g0:g1].unsqueeze(2).to_broadcast([NES, 32, NE]),
                        op=mybir.AluOpType.is_equal)
                    nc.vector.tensor_tensor(
                        out=tsel[:, t0:t1], in0=iseq[:, t0:t1],
                        in1=tokid_sb[:, t0:t1], op=mybir.AluOpType.mult)
                    nc.vector.tensor_reduce(
                        out=tid_f[:, g0:g1],
                        in_=tsel[:, t0:t1].rearrange("e (g t) -> e g t", t=NE),
                        axis=mybir.AxisListType.X, op=mybir.AluOpType.max)
                    # perm tiles for this chunk's 4 token tiles:
                    # iseq [es, tok] -> [tok, es] via PE transpose, then mask
                    # by group-line to [tok, es, gl]
                    for tt in range(4 * tc4, 4 * tc4 + 4):
                        psum_t = psC.tile([128, NES], f32, space="PSUM")
                        nc.tensor.transpose(
                            out=psum_t[:],
                            in_=iseq[:, tt * 128:(tt + 1) * 128],
                            identity=ident_sb[:])
                        if tt % 2 == 0:
                            nc.scalar.copy(
                                out=perm_all[:, tt, :, :],
                                in_=psum_t.unsqueeze(2).to_broadcast(
                                    [128, NES, GPT]))
                        else:
                            nc.vector.tensor_copy(
                                out=perm_all[:, tt, :, :],
                                in_=psum_t.unsqueeze(2).to_broadcast(
                                    [128, NES, GPT]))
                        eng = nc.gpsimd if tt % 2 == 0 else nc.vector
                        eng.tensor_tensor(
                            out=perm_all[:, tt, :, :],
                            in0=perm_all[:, tt, :, :],
                            in1=glmask_sb.unsqueeze(1).to_broadcast(
                                [128, NES, GPT]),
                            op=mybir.AluOpType.mult)
                # export routing table for the host-side combine
                tid_i32 = rt.tile([NES, G], i32)
                nc.vector.tensor_copy(out=tid_i32[:], in_=tid_f[:])
                nc.sync.dma_start(out=tids, in_=tid_i32[:])

            bc_cm.__exit__(None, None, None)

            # ---- phase D: dispatch + FFN, es-halves, software-pipelined ----
            # Down-proj of half 0 (DMA-heavy: stage3 writes) is interleaved
            # with dispatch of half 1 (PE-heavy, no DMA); up/down of half 1
            # then interleave into the down-half-0 stream so stage3 writes
            # start early and spread across the runtime.
            f1_r = f1w.rearrange("(k p) q -> p k q", p=128)
            with (
                tc.tile_pool(name="selp", bufs=1) as selp,
                tc.tile_pool(name="dcp", bufs=1) as dcp,
                tc.tile_pool(name="wp1", bufs=4) as wp1,
                tc.tile_pool(name="wp2", bufs=4) as wp2,
                tc.tile_pool(name="yp", bufs=3) as yp,
            ):
                sel_tiles = {}
                h_tiles = {}
                f1_tiles = {}
                f2_tiles = {}
                ecnt = [0]

                def ealt():
                    ecnt[0] += 1
                    return ecnt[0] % 2 == 0

                def emit_disp_chunk(psD, half, i):
                    # i in 0..31: (tt, kq) chunk of 4 k-tiles, N=256
                    tt, kq = divmod(i, 2)
                    e0 = half * EHALF
                    if i == 0:
                        sel_tiles[half] = selp.tile(
                            [128, KT, NTT, EHALF, GPT], bfl, tag="selT",
                            name=f"selT_{half}")
                    selT = sel_tiles[half]
                    psum_s = psD.tile([128, 4, 256], f32, space="PSUM",
                                      tag=f"d{i % 2}")
                    for kk in range(4):
                        k = kq * 4 + kk
                        nc.tensor.matmul(
                            psum_s[:, kk, :],
                            x_sb[:, tt, k * 128:(k + 1) * 128],
                            perm_all[:, tt, e0:e0 + EHALF, :],
                            start=True, stop=True)
                    dst = selT[:, kq * 4:(kq + 1) * 4, tt, :, :]
                    sc = psum_s.rearrange("p k (e g) -> p k e g", e=EHALF)
                    if ealt():
                        nc.vector.tensor_copy(out=dst, in_=sc)
                    else:
                        nc.scalar.copy(out=dst, in_=sc)

                def emit_up_pair(psH, half, j):
                    # j in 0..15: es pair within half, col-tiled 128x64
                    selT = sel_tiles[half]
                    if j == 0:
                        h_tiles[half] = dcp.tile([128, PPH, G], bfl, tag="h",
                                                 name=f"h_{half}")
                    h_all = h_tiles[half]
                    a8, jj = divmod(j, 4)
                    ag = 4 * half + a8             # global octet
                    if ag not in f1_tiles:
                        f1_sb = wp1.tile([128, KT, 8 * ESZ], bfl, tag="f1",
                                         name=f"f1_{ag}")
                        nc.sync.dma_start(
                            out=f1_sb[:],
                            in_=f1_r[:, :, ag * 8 * ESZ:(ag + 1) * 8 * ESZ])
                        f1_tiles[ag] = f1_sb
                    f1_sb = f1_tiles[ag]
                    jg = half * PPH + j            # global pair
                    psum_h = psH.tile([128, G], f32, space="PSUM")
                    el = jj * 2
                    for k in range(KT):
                        nc.tensor.matmul(
                            psum_h[0:64, :],
                            f1_sb[:, k, el * ESZ:(el + 1) * ESZ],
                            selT[:, k, :, a8 * 8 + el, :],
                            start=(k == 0), stop=(k == KT - 1),
                            tile_position=(0, 0))
                        nc.tensor.matmul(
                            psum_h[64:128, :],
                            f1_sb[:, k, (el + 1) * ESZ:(el + 2) * ESZ],
                            selT[:, k, :, a8 * 8 + el + 1, :],
                            start=(k == 0), stop=(k == KT - 1),
                            tile_position=(0, 64))
                    nc.scalar.activation(
                        out=h_all[:, j, :], in_=psum_h[:],
                        func=mybir.ActivationFunctionType.Relu,
                        bias=bias_sb[:, jg:jg + 1], scale=1.0)

                def emit_down_pair(psY, half, j):
                    # j in 0..15: es pair, row-tiled 64x128
                    h_all = h_tiles[half]
                    jg = half * PPH + j
                    ac, jj = divmod(jg, 4)
                    if ac not in f2_tiles:
                        f2_sb = wp2.tile([128, 4, DM], bfl, tag="f2",
                                         name=f"f2_{ac}")
                        nc.sync.dma_start(
                            out=f2_sb[:], in_=f2w[:, ac * 4:(ac + 1) * 4, :])
                        f2_tiles[ac] = f2_sb
                    f2_sb = f2_tiles[ac]
                    ps = [psY.tile([128, 512], f32, space="PSUM", tag=f"y{m}",
                                   name=f"py_{jg}_{m}")
                          for m in range(4)]
                    for n in range(2):
                        nc.tensor.matmul(
                            ps[n][:],
                            h_all[0:64, j, :],
                            f2_sb[0:64, jj, n * 512:(n + 1) * 512],
                            start=True, stop=True, tile_position=(0, 0))
                        nc.tensor.matmul(
                            ps[2 + n][:],
                            h_all[64:128, j, :],
                            f2_sb[64:128, jj, n * 512:(n + 1) * 512],
                            start=True, stop=True, tile_position=(64, 0))
                    y_sb = yp.tile([128, 2, DM], bfl)
                    for m in range(4):
                        dst = y_sb[:, m // 2, (m % 2) * 512:(m % 2 + 1) * 512]
                        if ealt():
                            nc.vector.tensor_copy(out=dst, in_=ps[m][:])
                        else:
                            nc.scalar.copy(out=dst, in_=ps[m][:])
                    # dense write: rows (es, g) for es = 2*jg, 2*jg+1
                    nc.sync.dma_start(
                        out=stage3[2 * jg * G:(2 * jg + 2) * G, :]
                        .rearrange("(e g) d -> g e d", g=G),
                        in_=y_sb[:])

                def prefetch_f1(ag):
                    f1_sb = wp1.tile([128, KT, 8 * ESZ], bfl, tag="f1",
                                     name=f"f1_{ag}")
                    nc.sync.dma_start(
                        out=f1_sb[:],
                        in_=f1_r[:, :, ag * 8 * ESZ:(ag + 1) * 8 * ESZ])
                    f1_tiles[ag] = f1_sb

                def prefetch_f2(ac):
                    f2_sb = wp2.tile([128, 4, DM], bfl, tag="f2",
                                     name=f"f2_{ac}")
                    nc.sync.dma_start(
                        out=f2_sb[:], in_=f2w[:, ac * 4:(ac + 1) * 4, :])
                    f2_tiles[ac] = f2_sb

                # schedule: psum pools scoped so concurrent phases fit 8 banks
                with tc.tile_pool(name="psD", bufs=1, space="PSUM") as psD:
                    for ag in range(4):
                        prefetch_f1(ag)       # f1 half-0 loads during dispatch
                    for i in range(32):
                        emit_disp_chunk(psD, 0, i)
                        if i == 8:
                            for ac in range(3):
                                prefetch_f2(ac)  # f2 heads load during dispatch
                    with tc.tile_pool(name="psH", bufs=2, space="PSUM") as psH:
                        for j in range(PPH):
                            emit_up_pair(psH, 0, j)
                    with tc.tile_pool(name="psY", bufs=1, space="PSUM") as psY:
                        for j in range(PPH):
                            emit_down_pair(psY, 0, j)
                            for i in range(2 * j, 2 * j + 2):
                                emit_disp_chunk(psD, 1, i)
                            if 4 <= j < 8:
                                # prefetch half-1 f1 octets: their wp1 slots
                                # freed once up half-0 octet j-4 was consumed
                                prefetch_f1(j)
                            if 10 <= j < 13:
                                prefetch_f2(j - 7)  # f2 chunks 3..5
                with tc.tile_pool(name="psH", bufs=4, space="PSUM") as psH:
                    for j in range(PPH):
                        emit_up_pair(psH, 1, j)
                with tc.tile_pool(name="psY", bufs=2, space="PSUM") as psY:
                    for j in range(PPH):
                        emit_down_pair(psY, 1, j)

    nc.compile()
    return nc


def _host_prep(x, controller, f1, f2, bias):
    """Returns (shared_map, per_core_maps)."""
    x = np.asarray(x, dtype=np.float32)
    c2 = np.ascontiguousarray(np.asarray(controller, np.float32).reshape(DM, NES))
    c2h = c2.astype(bf16)
    c2l = (c2 - c2h.astype(np.float32)).astype(bf16)
    f1w = np.ascontiguousarray(np.asarray(f1, np.float32).reshape(DM, NES * ESZ)).astype(bf16)
    # f2 stacked in es pairs: [(pair-parity f), pair, DM]
    f2p = np.asarray(f2, np.float32).reshape(NPAIR, 2, ESZ, DM)
    f2w = np.ascontiguousarray(f2p.transpose(1, 2, 0, 3).reshape(128, NPAIR, DM)).astype(bf16)
    b2 = np.asarray(bias, np.float32).reshape(NPAIR, 2, ESZ)
    bias2 = np.ascontiguousarray(b2.transpose(1, 2, 0).reshape(128, NPAIR))
    tie = np.linspace(0.0, 1e-6, NE, dtype=np.float32)
    tie512 = np.broadcast_to(np.tile(tie, 512 // NE), (NES, 512)).copy()
    tokid = np.broadcast_to(np.arange(SEQ, dtype=np.float32), (NES, SEQ)).copy()
    ident = np.eye(NES, dtype=np.float32)
    gl = (np.arange(128) // NE)[:, None] == np.arange(GPT)[None, :]
    glmask = np.ascontiguousarray(gl.astype(bf16))
    shared = dict(c2h=c2h, c2l=c2l, f1w=f1w, f2w=f2w, bias2=bias2,
                  tie512=tie512, tokid=tokid, ident=ident, glmask=glmask)
    per_core = []
    for b in range(NCORES):
        xb = x[b]
        xT = np.ascontiguousarray(xb.T)
        xTh = xT.astype(bf16)
        xTl = (xT - xTh.astype(np.float32)).astype(bf16)
        per_core.append(dict(
            xTh=xTh, xTl=xTl,
            xbf=np.ascontiguousarray(
                xb.astype(bf16).reshape(NTT, 128, DM).transpose(1, 0, 2)),
        ))
    return shared, per_core


def _run(inputs, trace=False, tmpdir=None, trace_cores=None):
    if "nc" not in _CACHE:
        _CACHE["nc"] = _build_program()
    nc = _CACHE["nc"]
    shared, per_core = _host_prep(
        inputs["x"], inputs["controller"], inputs["f1"], inputs["f2"],
        inputs["bias"])
    in_maps = [dict(shared, **pc) for pc in per_core]
    res = run_bass_kernel_spmd(
        nc, in_maps, core_ids=list(range(NCORES)), trace=trace, tmpdir=tmpdir,
        trace_cores=trace_cores)
    out = np.zeros((NCORES, SEQ, DM), dtype=np.float32)
    for b in range(NCORES):
        st = np.asarray(res.results[b]["stage3"]).astype(np.float32)
        tid = np.asarray(res.results[b]["tids"]).reshape(-1)  # [es*G] token ids
        rows = tid.reshape(NES, G)
        # stage3 row es*G + g holds y for (es, group g)
        np.add.at(out[b], rows.reshape(-1), st)
    return out, res


def kernel(**inputs) -> np.ndarray:
    out, _ = _run(inputs)
    return out


# revision 61
# speedup vs baseline: 1.2023x; 1.2023x over previous
"""Trainium2 Bass kernel for nn_BatchSplitFF (expert-choice MoE routing + FFN).

Strategy (data-parallel over batch, 1 batch per NeuronCore, 8 cores):
  - fp32 logits on PE in [es, tok] orientation, N=512 token chunks (routing
    must match the fp32 reference argmax; bf16 logits would flip decisions).
  - routing on DVE: group max -> one-hot iseq (== perm matrix in [es, tok]
    orientation) -> token ids exported for the host-side combine.
  - dispatch ON-CHIP via PE one-hot matmuls (no DMA gather: SWDGE descriptor
    overhead ~150us/queue dominated the old design): per 128-token tile,
    selT[dm, slot] = x_tile^T @ perm_tile. perm tiles are built on DVE from
    PE-transposed iseq slices. Two es-halves keep selT at 8MB in SBUF.
  - expert FFN in bf16 on PE with array tiling: up-proj packs es pairs into
    128x64 column tiles (2x), down-proj packs es pairs into 64x128 row tiles
    (2x). relu/bias on ScalarE.
  - un-permute: y rows are written DENSELY (one row per (es, g) contribution)
    with contiguous DMAs; the routing table (token id per contribution) is
    exported, and the host does the final scatter-add combine in fp32
    (8.4M adds = 0.04% of the FLOPs). dma_scatter_add loses duplicate-row
    updates (verified on HW) and on-chip combine requires a y transpose that
    costs as much as the dense write saves.
Host side only reshapes/casts/transposes inputs and combines the output.
"""

import numpy as np
import ml_dtypes

import concourse.bass as bass
import concourse.mybir as mybir
import concourse.tile as tile
from concourse import bacc
from concourse.bass_utils import run_bass_kernel_spmd

bf16 = ml_dtypes.bfloat16
f32 = mybir.dt.float32
f32r = mybir.dt.float32r
bfl = mybir.dt.bfloat16
i32 = mybir.dt.int32

DM, NE, ES, ESZ = 1024, 16, 4, 64
NES = NE * ES            # 64 (e,s) expert pairs
SEQ = 2048
G = SEQ // NE            # 128 groups per core
KT = DM // 128           # 8 contraction tiles
NCORES = 8
NTT = SEQ // 128         # 16 token tiles (8 groups each)
GPT = 8                  # groups per token tile
EHALF = NES // 2         # 32 es per dispatch half
NPAIR = NES // 2         # 32 es pairs
PPH = NPAIR // 2         # 16 pairs per half

_CACHE = {}


def _build_program():
    nc = bacc.Bacc("TRN2", target_bir_lowering=False, debug=False,
                   enable_asserts=False, num_devices=1)

    xTh = nc.dram_tensor("xTh", [DM, SEQ], bfl, kind="ExternalInput").ap()
    xTl = nc.dram_tensor("xTl", [DM, SEQ], bfl, kind="ExternalInput").ap()
    # natural-layout bf16 x wrapped [(r p) d -> p r d]: token t at
    # partition t%128, rank t//128 -> x_sb[:, tt, :] is a 128-token tile
    xbf = nc.dram_tensor("xbf", [128, NTT, DM], bfl, kind="ExternalInput").ap()
    c2h = nc.dram_tensor("c2h", [DM, NES], bfl, kind="ExternalInput").ap()
    c2l = nc.dram_tensor("c2l", [DM, NES], bfl, kind="ExternalInput").ap()
    f1w = nc.dram_tensor("f1w", [DM, NES * ESZ], bfl, kind="ExternalInput").ap()
    # f2 stacked in es pairs: [2*64 f, 32 pairs, DM]
    f2w = nc.dram_tensor("f2w", [128, NPAIR, DM], bfl, kind="ExternalInput").ap()
    # bias stacked in es pairs: [2*64 f, 32 pairs]
    bias2 = nc.dram_tensor("bias2", [128, NPAIR], f32, kind="ExternalInput").ap()
    tie512 = nc.dram_tensor("tie512", [NES, 512], f32, kind="ExternalInput").ap()
    tokid = nc.dram_tensor("tokid", [NES, SEQ], f32, kind="ExternalInput").ap()
    ident = nc.dram_tensor("ident", [NES, NES], f32, kind="ExternalInput").ap()
    # glmask[t%128, gl] = 1.0 iff (t%128)//16 == gl
    glmask = nc.dram_tensor("glmask", [128, GPT], bfl, kind="ExternalInput").ap()
    stage3 = nc.dram_tensor("stage3", [NES * G, DM], bfl,
                            kind="ExternalOutput").ap()
    tids = nc.dram_tensor("tids", [NES, G], i32, kind="ExternalOutput").ap()

    with tile.TileContext(nc) as tc:
        with (
            tc.tile_pool(name="consts", bufs=1) as consts,
            tc.tile_pool(name="route", bufs=1) as route,
        ):
            # ---- constants into SBUF ----
            ch_sb = consts.tile([128, KT, NES], bfl)
            nc.sync.dma_start(out=ch_sb[:], in_=c2h.rearrange("(k p) e -> p k e", p=128))
            cl_sb = consts.tile([128, KT, NES], bfl)
            nc.sync.dma_start(out=cl_sb[:], in_=c2l.rearrange("(k p) e -> p k e", p=128))
            tie_sb = consts.tile([NES, 512], f32)
            nc.sync.dma_start(out=tie_sb[:], in_=tie512)
            bias_sb = consts.tile([128, NPAIR], f32)
            tokid_sb = consts.tile([NES, SEQ], f32)
            ident_sb = consts.tile([NES, NES], f32)
            glmask_sb = consts.tile([128, GPT], bfl)
            x_sb = consts.tile([128, NTT, DM], bfl)

            # ---- phase B: fp32 logits, [es, tok], 512-token chunks ----
            # PE warmup: ~5us of dummy matmuls during the initial DMA wait
            # releases the HAM clock gate so real matmuls run at 2.4 GHz
            with tc.tile_pool(name="warm", bufs=1, space="PSUM") as warm:
                wps = warm.tile([NES, 512], f32, space="PSUM")
                wrhs = ch_sb.rearrange("p k e -> p (k e)")
                for _ in range(12):
                    nc.tensor.matmul(wps[:], ch_sb[:, 0, :], wrhs,
                                     start=True, stop=True)

            bc_cm = tc.tile_pool(name="bc", bufs=1)
            bc = bc_cm.__enter__()
            logits_sb = bc.tile([NES, SEQ], f32)
            iseq = bc.tile([NES, SEQ], f32)
            tsel = bc.tile([NES, SEQ], f32)
            perm_all = route.tile([128, NTT, NES, GPT], bfl)
            with (
                tc.tile_pool(name="xtp", bufs=3) as xtp,
                tc.tile_pool(name="rt", bufs=1) as rt,
                tc.tile_pool(name="psB", bufs=2, space="PSUM") as psB,
                tc.tile_pool(name="psC", bufs=4, space="PSUM") as psC,
            ):
                gmax = rt.tile([NES, G], f32)
                tid_f = rt.tile([NES, G], f32)
                xTh_r = xTh.rearrange("(k p) t -> p k t", p=128)
                xTl_r = xTl.rearrange("(k p) t -> p k t", p=128)
                xt_tiles = []
                for tc4 in range(SEQ // 512):
                    xt_t = xtp.tile([128, 2, KT, 512], bfl, tag="xt",
                                    name=f"xt_{tc4}")
                    nc.sync.dma_start(
                        out=xt_t[:, 0, :, :],
                        in_=xTh_r[:, :, tc4 * 512:(tc4 + 1) * 512])
                    nc.sync.dma_start(
                        out=xt_t[:, 1, :, :],
                        in_=xTl_r[:, :, tc4 * 512:(tc4 + 1) * 512])
                    xt_tiles.append(xt_t)
                    if tc4 == 0:
                        # deferred consts: queued after the first logits chunk
                        nc.sync.dma_start(out=tokid_sb[:], in_=tokid)
                        nc.sync.dma_start(out=ident_sb[:], in_=ident)
                        nc.sync.dma_start(out=glmask_sb[:], in_=glmask)
                        nc.sync.dma_start(out=bias_sb[:], in_=bias2)
                    if tc4 == 3:
                        # x_sb (dispatch source) queued after all logits
                        # chunks; needed only when dispatch starts
                        nc.sync.dma_start(out=x_sb[:], in_=xbf)
                # fused logits + routing, per 512-token chunk (32 groups)
                for tc4 in range(SEQ // 512):
                    xt_t = xt_tiles[tc4]
                    t0, t1 = tc4 * 512, (tc4 + 1) * 512
                    g0, g1 = tc4 * 32, (tc4 + 1) * 32
                    psum_l = psB.tile([NES, 512], f32, space="PSUM")
                    # fp32-split logits: xh*ch + xh*cl + xl*ch (bf16 full rate)
                    for k in range(KT):
                        nc.tensor.matmul(psum_l[:], ch_sb[:, k, :],
                                         xt_t[:, 0, k, :],
                                         start=(k == 0), stop=False)
                        nc.tensor.matmul(psum_l[:], cl_sb[:, k, :],
                                         xt_t[:, 0, k, :],
                                         start=False, stop=False)
                        nc.tensor.matmul(psum_l[:], ch_sb[:, k, :],
                                         xt_t[:, 1, k, :],
                                         start=False, stop=(k == KT - 1))
                    # add tiebreak while copying PSUM -> SBUF
                    nc.vector.tensor_tensor(
                        out=logits_sb[:, t0:t1],
                        in0=psum_l[:], in1=tie_sb[:],
                        op=mybir.AluOpType.add)
                    # routing for this chunk's 32 groups on DVE
                    lg = logits_sb[:, t0:t1].rearrange("e (g t) -> e g t", t=NE)
                    nc.vector.tensor_reduce(
                        out=gmax[:, g0:g1], in_=lg,
                        axis=mybir.AxisListType.X, op=mybir.AluOpType.max)
                    nc.vector.tensor_tensor(
                        out=iseq[:, t0:t1].rearrange("e (g t) -> e g t", t=NE),
                        in0=lg,
                        in1=gmax[:, g0:g1].unsqueeze(2).to_broadcast([NES, 32, NE]),
                        op=mybir.AluOpType.is_equal)
                    nc.vector.tensor_tensor(
                        out=tsel[:, t0:t1], in0=iseq[:, t0:t1],
                        in1=tokid_sb[:, t0:t1], op=mybir.AluOpType.mult)
                    nc.vector.tensor_reduce(
                        out=tid_f[:, g0:g1],
                        in_=tsel[:, t0:t1].rearrange("e (g t) -> e g t", t=NE),
                        axis=mybir.AxisListType.X, op=mybir.AluOpType.max)
                    # perm tiles for this chunk's 4 token tiles:
                    # iseq [es, tok] -> [tok, es] via PE transpose, then mask
                    # by group-line to [tok, es, gl]
                    for tt in range(4 * tc4, 4 * tc4 + 4):
                        psum_t = psC.tile([128, NES], f32, space="PSUM")
                        nc.tensor.transpose(
                            out=psum_t[:],
                            in_=iseq[:, tt * 128:(tt + 1) * 128],
                            identity=ident_sb[:])
                        if tt % 2 == 0:
                            nc.scalar.copy(
                                out=perm_all[:, tt, :, :],
                                in_=psum_t.unsqueeze(2).to_broadcast(
                                    [128, NES, GPT]))
                        else:
                            nc.vector.tensor_copy(
                                out=perm_all[:, tt, :, :],
                                in_=psum_t.unsqueeze(2).to_broadcast(
                                    [128, NES, GPT]))
                        eng = nc.gpsimd if tt % 2 == 0 else nc.vector
                        eng.tensor_tensor(
                            out=perm_all[:, tt, :, :],
                            in0=perm_all[:, tt, :, :],
                            in1=glmask_sb.unsqueeze(1).to_broadcast(
                                [128, NES, GPT]),
                            op=mybir.AluOpType.mult)
                # export routing table for the host-side combine
                tid_i32 = rt.tile([NES, G], i32)
                nc.vector.tensor_copy(out=tid_i32[:], in_=tid_f[:])
                nc.sync.dma_start(out=tids, in_=tid_i32[:])

            bc_cm.__exit__(None, None, None)

            # ---- phase D: dispatch + FFN, es-halves, software-pipelined ----
            # Down-proj of half 0 (DMA-heavy: stage3 writes) is interleaved
            # with dispatch of half 1 (PE-heavy, no DMA); up/down of half 1
            # then interleave into the down-half-0 stream so stage3 writes
            # start early and spread across the runtime.
            f1_r = f1w.rearrange("(k p) q -> p k q", p=128)
            with (
                tc.tile_pool(name="selp", bufs=1) as selp,
                tc.tile_pool(name="dcp", bufs=1) as dcp,
                tc.tile_pool(name="wp1", bufs=4) as wp1,
                tc.tile_pool(name="wp2", bufs=4) as wp2,
                tc.tile_pool(name="yp", bufs=3) as yp,
            ):
                sel_tiles = {}
                h_tiles = {}
                f1_tiles = {}
                f2_tiles = {}
                ecnt = [0]

                def ealt():
                    ecnt[0] += 1
                    return ecnt[0] % 2 == 0

                def emit_disp_chunk(psD, half, i):
                    # i in 0..31: (tt, kq) chunk of 4 k-tiles, N=256
                    tt, kq = divmod(i, 2)
                    e0 = half * EHALF
                    if i == 0:
                        sel_tiles[half] = selp.tile(
                            [128, KT, NTT, EHALF, GPT], bfl, tag="selT",
                            name=f"selT_{half}")
                    selT = sel_tiles[half]
                    psum_s = psD.tile([128, 4, 256], f32, space="PSUM",
                                      tag=f"d{i % 2}")
                    for kk in range(4):
                        k = kq * 4 + kk
                        nc.tensor.matmul(
                            psum_s[:, kk, :],
                            x_sb[:, tt, k * 128:(k + 1) * 128],
                            perm_all[:, tt, e0:e0 + EHALF, :],
                            start=True, stop=True)
                    dst = selT[:, kq * 4:(kq + 1) * 4, tt, :, :]
                    sc = psum_s.rearrange("p k (e g) -> p k e g", e=EHALF)
                    if ealt():
                        nc.vector.tensor_copy(out=dst, in_=sc)
                    else:
                        nc.scalar.copy(out=dst, in_=sc)

                def emit_up_pair(psH, half, j):
                    # j in 0..15: es pair within half, col-tiled 128x64
                    selT = sel_tiles[half]
                    if j == 0:
                        h_tiles[half] = dcp.tile([128, PPH, G], bfl, tag="h",
                                                 name=f"h_{half}")
                    h_all = h_tiles[half]
                    a8, jj = divmod(j, 4)
                    ag = 4 * half + a8             # global octet
                    if ag not in f1_tiles:
                        f1_sb = wp1.tile([128, KT, 8 * ESZ], bfl, tag="f1",
                                         name=f"f1_{ag}")
                        nc.sync.dma_start(
                            out=f1_sb[:],
                            in_=f1_r[:, :, ag * 8 * ESZ:(ag + 1) * 8 * ESZ])
                        f1_tiles[ag] = f1_sb
                    f1_sb = f1_tiles[ag]
                    jg = half * PPH + j            # global pair
                    psum_h = psH.tile([128, G], f32, space="PSUM")
                    el = jj * 2
                    for k in range(KT):
                        nc.tensor.matmul(
                            psum_h[0:64, :],
                            f1_sb[:, k, el * ESZ:(el + 1) * ESZ],
                            selT[:, k, :, a8 * 8 + el, :],
                            start=(k == 0), stop=(k == KT - 1),
                            tile_position=(0, 0))
                        nc.tensor.matmul(
                            psum_h[64:128, :],
                            f1_sb[:, k, (el + 1) * ESZ:(el + 2) * ESZ],
                            selT[:, k, :, a8 * 8 + el + 1, :],
                            start=(k == 0), stop=(k == KT - 1),
                            tile_position=(0, 64))
                    nc.scalar.activation(
                        out=h_all[:, j, :], in_=psum_h[:],
                        func=mybir.ActivationFunctionType.Relu,
                        bias=bias_sb[:, jg:jg + 1], scale=1.0)

                def emit_down_pair(psY, half, j):
                    # j in 0..15: es pair, row-tiled 64x128
                    h_all = h_tiles[half]
                    jg = half * PPH + j
                    ac, jj = divmod(jg, 4)
                    if ac not in f2_tiles:
                        f2_sb = wp2.tile([128, 4, DM], bfl, tag="f2",
                                         name=f"f2_{ac}")
                        nc.sync.dma_start(
                            out=f2_sb[:], in_=f2w[:, ac * 4:(ac + 1) * 4, :])
                        f2_tiles[ac] = f2_sb
                    f2_sb = f2_tiles[ac]
                    ps = [psY.tile([128, 512], f32, space="PSUM", tag=f"y{m}",
                                   name=f"py_{jg}_{m}")
                          for m in range(4)]
                    for n in range(2):
                        nc.tensor.matmul(
                            ps[n][:],
                            h_all[0:64, j, :],
                            f2_sb[0:64, jj, n * 512:(n + 1) * 512],
                            start=True, stop=True, tile_position=(0, 0))
                        nc.tensor.matmul(
                            ps[2 + n][:],
                            h_all[64:128, j, :],
                            f2_sb[64:128, jj, n * 512:(n + 1) * 512],
                            start=True, stop=True, tile_position=(64, 0))
                    y_sb = yp.tile([128, 2, DM], bfl)
                    for m in range(4):
                        dst = y_sb[:, m // 2, (m % 2) * 512:(m % 2 + 1) * 512]
                        if ealt():
                            nc.vector.tensor_copy(out=dst, in_=ps[m][:])
                        else:
                            nc.scalar.copy(out=dst, in_=ps[m][:])
                    # dense write: rows (es, g) for es = 2*jg, 2*jg+1
                    nc.sync.dma_start(
                        out=stage3[2 * jg * G:(2 * jg + 2) * G, :]
                        .rearrange("(e g) d -> g e d", g=G),
                        in_=y_sb[:])

                def prefetch_f1(ag):
                    f1_sb = wp1.tile([128, KT, 8 * ESZ], bfl, tag="f1",
                                     name=f"f1_{ag}")
                    nc.sync.dma_start(
                        out=f1_sb[:],
                        in_=f1_r[:, :, ag * 8 * ESZ:(ag + 1) * 8 * ESZ])
                    f1_tiles[ag] = f1_sb

                def prefetch_f2(ac):
                    f2_sb = wp2.tile([128, 4, DM], bfl, tag="f2",
                                     name=f"f2_{ac}")
                    nc.sync.dma_start(
                        out=f2_sb[:], in_=f2w[:, ac * 4:(ac + 1) * 4, :])
                    f2_tiles[ac] = f2_sb

                # schedule: psum pools scoped so concurrent phases fit 8 banks
                with tc.tile_pool(name="psD", bufs=1, space="PSUM") as psD:
                    for ag in range(4):
                        prefetch_f1(ag)       # f1 half-0 loads during dispatch
                    for i in range(32):
                        emit_disp_chunk(psD, 0, i)
                        if i == 8:
                            for ac in range(3):
                                prefetch_f2(ac)  # f2 heads load during dispatch
                    with tc.tile_pool(name="psH", bufs=2, space="PSUM") as psH:
                        for j in range(PPH):
                            emit_up_pair(psH, 0, j)
                    with tc.tile_pool(name="psY", bufs=1, space="PSUM") as psY:
                        for j in range(PPH):
                            emit_down_pair(psY, 0, j)
                            for i in range(2 * j, 2 * j + 2):
                                emit_disp_chunk(psD, 1, i)
                with tc.tile_pool(name="psH", bufs=4, space="PSUM") as psH:
                    for j in range(PPH):
                        emit_up_pair(psH, 1, j)
                with tc.tile_pool(name="psY", bufs=2, space="PSUM") as psY:
                    for j in range(PPH):
                        emit_down_pair(psY, 1, j)

    nc.compile()
    return nc


def _host_prep(x, controller, f1, f2, bias):
    """Returns (shared_map, per_core_maps)."""
    x = np.asarray(x, dtype=np.float32)
    c2 = np.ascontiguousarray(np.asarray(controller, np.float32).reshape(DM, NES))
    c2h = c2.astype(bf16)
    c2l = (c2 - c2h.astype(np.float32)).astype(bf16)
    f1w = np.ascontiguousarray(np.asarray(f1, np.float32).reshape(DM, NES * ESZ)).astype(bf16)
    # f2 stacked in es pairs: [(pair-parity f), pair, DM]
    f2p = np.asarray(f2, np.float32).reshape(NPAIR, 2, ESZ, DM)
    f2w = np.ascontiguousarray(f2p.transpose(1, 2, 0, 3).reshape(128, NPAIR, DM)).astype(bf16)
    b2 = np.asarray(bias, np.float32).reshape(NPAIR, 2, ESZ)
    bias2 = np.ascontiguousarray(b2.transpose(1, 2, 0).reshape(128, NPAIR))
    tie = np.linspace(0.0, 1e-6, NE, dtype=np.float32)
    tie512 = np.broadcast_to(np.tile(tie, 512 // NE), (NES, 512)).copy()
    tokid = np.broadcast_to(np.arange(SEQ, dtype=np.float32), (NES, SEQ)).copy()
    ident = np.eye(NES, dtype=np.float32)
    gl = (np.arange(128) // NE)[:, None] == np.arange(GPT)[None, :]
    glmask = np.ascontiguousarray(gl.astype(bf16))
    shared = dict(c2h=c2h, c2l=c2l, f1w=f1w, f2w=f2w, bias2=bias2,
                  tie512=tie512, tokid=tokid, ident=ident, glmask=glmask)
    per_core = []
    for b in range(NCORES):
        xb = x[b]
        xT = np.ascontiguousarray(xb.T)
        xTh = xT.astype(bf16)
        xTl = (xT - xTh.astype(np.float32)).astype(bf16)
        per_core.append(dict(
            xTh=xTh, xTl=xTl,
            xbf=np.ascontiguousarray(
                xb.astype(bf16).reshape(NTT, 128, DM).transpose(1, 0, 2)),
        ))
    return shared, per_core


def _run(inputs, trace=False, tmpdir=None, trace_cores=None):
    if "nc" not in _CACHE:
        _CACHE["nc"] = _build_program()
    nc = _CACHE["nc"]
    shared, per_core = _host_prep(
        inputs["x"], inputs["controller"], inputs["f1"], inputs["f2"],
        inputs["bias"])
    in_maps = [dict(shared, **pc) for pc in per_core]
    res = run_bass_kernel_spmd(
        nc, in_maps, core_ids=list(range(NCORES)), trace=trace, tmpdir=tmpdir,
        trace_cores=trace_cores)
    out = np.zeros((NCORES, SEQ, DM), dtype=np.float32)
    for b in range(NCORES):
        st = np.asarray(res.results[b]["stage3"]).astype(np.float32)
        tid = np.asarray(res.results[b]["tids"]).reshape(-1)  # [es*G] token ids
        rows = tid.reshape(NES, G)
        # stage3 row es*G + g holds y for (es, group g)
        np.add.at(out[b], rows.reshape(-1), st)
    return out, res


def kernel(**inputs) -> np.ndarray:
    out, _ = _run(inputs)
    return out
